# revision 21
# baseline (speedup 1.0000x reference)
"""Depthwise Conv1d (C=128, K=3, stride=1, pad=1) Trainium2 Bass kernel.

Layout: partitions = channels (C=128 exactly matches SBUF partitions).
Sharding: data-parallel over batch — 32 images / 8 cores = 4 images/core.
Per 4096-wide chunk (default bf16_tt path):
    ACT : t  = w1 * x_center + bias   (activation, f32 in -> bf16 out)
    ACT : p2 = w2 * x_right           (activation, f32 in -> bf16 out)
    DVE : u  = t + p2                 (tensor_tensor, all-bf16 -> 2x perf mode)
    DVE : res= (x_left * w0) + u      (scalar_tensor_tensor, f32 out)
    (every 3rd chunk's tensor_tensor runs on Pool/GpSimd instead of DVE)
bf16 intermediates round three of the four terms (~4e-3 rel error, well
inside the 2e-2 gate) and cut DVE's per-chunk time ~25% plus offload to
Pool, so compute stays off the critical path.
The kernel is HBM-bandwidth bound (~33.6 MB/core mandatory traffic;
measured sustained ~360-430 GB/s/core -> ~94us floor + ~9us NEFF
preamble). Stores issue on the scalar HWDGE ring one chunk late
(store_skew=1) so their semaphore wait is already satisfied and never
head-of-line-blocks the next activation; the final row tapers to small
tiles so the tail compute+store chain adds little to the DMA span.
"""

import numpy as np

import concourse.bacc as bacc
import concourse.mybir as mybir
import concourse.tile as tile
from concourse import bass_utils

B, C, L, K = 32, 128, 8192, 3
NCORES = 8
BPC = B // NCORES  # images per core

TILE_N = 4096
BUFS_IN = 4
BUFS_MID = 4
BUFS_ACC = 2
SUB_N = 4096

_nc_cache = {}


def _row_widths(bi, tile_n, taper):
    """Tile widths for image row bi (must sum to L)."""
    head = [512, 512, 1024, 2048] if taper >= 2 and bi == 0 else []
    # shrink the final tiles so the tail dependency chain
    # (last load -> compute -> last store) is short
    tail = [2048, 1024, 512, 512] if taper and bi == BPC - 1 else []
    body = L - sum(head) - sum(tail)
    if body % tile_n:
        tail = [body % tile_n] + tail
        body -= body % tile_n
    widths = head + [tile_n] * (body // tile_n) + tail
    assert sum(widths) == L, widths
    return widths


def _build_nc(
    tile_n=TILE_N,
    bufs_in=BUFS_IN,
    bufs_mid=BUFS_MID,
    bufs_acc=BUFS_ACC,
    store_on_scalar=1,
    store_on_gpsimd=0,
    loads_alternate=0,
    store_skew=1,
    taper=1,
    repeat=1,
    memset_on_gpsimd=0,
    gpsimd_every=0,
    const_on_scalar=0,
    const_late=1,
    sub_n=SUB_N,
    bf16_tt=1,
    pool_tt_every=3,
    mix=0,
    bufs_t=3,
    bufs_p2=3,
    bufs_u=2,
    ndev=1,
):
    f32 = mybir.dt.float32
    nc = bacc.Bacc(
        "TRN2",
        target_bir_lowering=False,
        debug=False,
        enable_asserts=False,
        num_devices=ndev,
    )
    x = nc.dram_tensor("x", [BPC, C, L], f32, kind="ExternalInput").ap()
    w = nc.dram_tensor("w", [C, K], f32, kind="ExternalInput").ap()
    b = nc.dram_tensor("b", [C, 1], f32, kind="ExternalInput").ap()
    y = nc.dram_tensor("y", [BPC, C, L], f32, kind="ExternalOutput").ap()

    mult = mybir.AluOpType.mult
    add = mybir.AluOpType.add
    ident = mybir.ActivationFunctionType.Identity

    with tile.TileContext(nc) as tc:
        with (
            tc.tile_pool(name="const", bufs=1) as cpool,
            tc.tile_pool(name="work", bufs=1) as pool,
        ):
            wtile = cpool.tile([C, K], f32)
            btile = cpool.tile([C, 1], f32)
            const_eng = nc.scalar if const_on_scalar else nc.sync
            consts_emitted = [False]

            def _emit_consts():
                if not consts_emitted[0]:
                    const_eng.dma_start(out=wtile[:, :], in_=w)
                    const_eng.dma_start(out=btile[:, :], in_=b)
                    consts_emitted[0] = True

            if not const_late:
                _emit_consts()

            store_eng = nc.scalar if store_on_scalar else nc.sync
            if store_on_gpsimd:
                store_eng = nc.gpsimd
            memset_eng = nc.gpsimd if memset_on_gpsimd else nc.vector
            it = 0
            for rep in range(repeat):
              # pending (data_view, dram_view) stores, issued `store_skew`
              # chunks late so their semaphore wait is already satisfied
              # when the store ring reaches them
              pending = []
              for bi in range(BPC):
                l0 = 0
                for n in _row_widths(bi, tile_n, taper):
                    load_eng = (
                        nc.scalar if loads_alternate and (it % 2) else nc.sync
                    )
                    # input halo range [l0-1, l0+n+1) clipped to [0, L)
                    lo, hi = l0 - 1, l0 + n + 1
                    src_lo, src_hi = max(lo, 0), min(hi, L)
                    dst = src_lo - lo

                    xin = pool.tile([C, tile_n + 2], f32, tag="xin", bufs=bufs_in)
                    if lo < 0:
                        memset_eng.memset(xin[:, 0:1], 0.0)
                    if hi > L:
                        memset_eng.memset(xin[:, n + 1 : n + 2], 0.0)
                    load_eng.dma_start(
                        out=xin[:, dst : dst + (src_hi - src_lo)],
                        in_=x[bi, :, src_lo:src_hi],
                    )
                    _emit_consts()

                    stt_eng = (
                        nc.gpsimd
                        if gpsimd_every and (it % gpsimd_every == gpsimd_every - 1)
                        else nc.vector
                    )
                    # compute+store in sub_n-wide chunks (loads stay tile_n
                    # wide) to shorten the compute-to-store latency per byte
                    step = sub_n if sub_n and sub_n < n else n
                    for s0 in range(0, n, step):
                        sn = min(step, n - s0)
                        mid = pool.tile([C, step], f32, tag="mid", bufs=bufs_mid)
                        if bf16_tt:
                            # ACT computes two taps in bf16; DVE combines
                            # them with a 2x-perf-mode bf16 tensor_tensor,
                            # then one stt adds the last tap in f32. This
                            # halves DVE's per-chunk time vs two f32 stt.
                            bf16 = mybir.dt.bfloat16
                            t = pool.tile([C, step], bf16, tag="t", bufs=bufs_t)
                            p2 = pool.tile([C, step], bf16, tag="p2", bufs=bufs_p2)
                            u = pool.tile([C, step], bf16, tag="u", bufs=bufs_u)
                            nc.scalar.activation(
                                t[:, 0:sn],
                                xin[:, s0 + 1 : s0 + sn + 1],
                                ident,
                                bias=btile[:, 0:1],
                                scale=wtile[:, 1:2],
                            )
                            nc.scalar.activation(
                                p2[:, 0:sn],
                                xin[:, s0 + 2 : s0 + sn + 2],
                                ident,
                                scale=wtile[:, 2:3],
                            )
                            tt_eng = (
                                nc.gpsimd
                                if pool_tt_every
                                and (it % pool_tt_every == pool_tt_every - 1)
                                else nc.vector
                            )
                            tt_eng.tensor_tensor(
                                u[:, 0:sn], t[:, 0:sn], p2[:, 0:sn], add
                            )
                            nc.vector.scalar_tensor_tensor(
                                mid[:, 0:sn], xin[:, s0 : s0 + sn],
                                wtile[:, 0:1], u[:, 0:sn], mult, add
                            )
                        else:
                            acc = pool.tile(
                                [C, step], f32, tag="acc", bufs=bufs_acc
                            )
                            nc.scalar.activation(
                                mid[:, 0:sn],
                                xin[:, s0 + 1 : s0 + sn + 1],
                                ident,
                                bias=btile[:, 0:1],
                                scale=wtile[:, 1:2],
                            )
                            stt_eng.scalar_tensor_tensor(
                                acc[:, 0:sn], xin[:, s0 : s0 + sn],
                                wtile[:, 0:1], mid[:, 0:sn], mult, add
                            )
                            stt_eng.scalar_tensor_tensor(
                                mid[:, 0:sn], xin[:, s0 + 2 : s0 + sn + 2],
                                wtile[:, 2:3], acc[:, 0:sn], mult, add
                            )
                        pending.append(
                            (mid[:, 0:sn], y[bi, :, l0 + s0 : l0 + s0 + sn])
                        )
                        while len(pending) > store_skew:
                            src, dst = pending.pop(0)
                            store_eng.dma_start(out=dst, in_=src)
                    l0 += n
                    it += 1
              for src, dst in pending:
                  store_eng.dma_start(out=dst, in_=src)

    nc.compile()
    return nc


def _get_nc(**kw):
    key = tuple(sorted(kw.items()))
    if key not in _nc_cache:
        _nc_cache[key] = _build_nc(**kw)
    return _nc_cache[key]


def kernel_with_results(inputs, weight, bias, trace=False, **build_kw):
    x = np.ascontiguousarray(inputs, dtype=np.float32)
    w = np.ascontiguousarray(weight, dtype=np.float32)
    b = np.ascontiguousarray(bias, dtype=np.float32).reshape(C, 1)
    assert x.shape == (B, C, L), x.shape
    nc = _get_nc(**build_kw)
    in_maps = [
        {"x": x[i * BPC : (i + 1) * BPC], "w": w, "b": b} for i in range(NCORES)
    ]
    res = bass_utils.run_bass_kernel_spmd(
        nc, in_maps, core_ids=list(range(NCORES)), trace=trace
    )
    out = np.concatenate([r["y"] for r in res.results], axis=0)
    return out, res


def kernel(inputs, weight, bias):
    out, _ = kernel_with_results(inputs, weight, bias)
    return out



# revision 26
# speedup vs baseline: 1.1364x; 1.1364x over previous
"""Depthwise Conv1d (C=128, K=3, stride=1, pad=1) Trainium2 Bass kernel.

Layout: partitions = channels (C=128 exactly matches SBUF partitions).
Sharding: data-parallel over batch — 32 images / 8 cores = 4 images/core.
Per 4096-wide chunk (default bf16_tt path):
    ACT : t  = w1 * x_center + bias   (activation, f32 in -> bf16 out)
    ACT : p2 = w2 * x_right           (activation, f32 in -> bf16 out)
    DVE : u  = t + p2                 (tensor_tensor, all-bf16 -> 2x perf mode)
    DVE : res= (x_left * w0) + u      (scalar_tensor_tensor, f32 out)
    (every 3rd chunk's tensor_tensor runs on Pool/GpSimd instead of DVE)
bf16 intermediates round three of the four terms (~4e-3 rel error, well
inside the 2e-2 gate) and cut DVE's per-chunk time ~25% plus offload to
Pool, so compute stays off the critical path.
The kernel is HBM-bandwidth bound (~33.6 MB/core mandatory traffic;
measured sustained ~360-430 GB/s/core -> ~94us floor + ~9us NEFF
preamble). Stores issue on the scalar HWDGE ring two chunks late
(store_skew=2) so their semaphore wait is already satisfied and never
head-of-line-blocks the next activation; the final row tapers to small
tiles so the tail compute+store chain adds little to the DMA span.
"""

import numpy as np

import concourse.bacc as bacc
import concourse.mybir as mybir
import concourse.tile as tile
from concourse import bass_utils

B, C, L, K = 32, 128, 8192, 3
NCORES = 8
BPC = B // NCORES  # images per core

TILE_N = 4096
BUFS_IN = 5
BUFS_MID = 3
BUFS_ACC = 2
SUB_N = 4096

_nc_cache = {}


def _row_widths(bi, tile_n, taper):
    """Tile widths for image row bi (must sum to L)."""
    head = [512, 512, 1024, 2048] if taper >= 2 and bi == 0 else []
    # shrink the final tiles so the tail dependency chain
    # (last load -> compute -> last store) is short
    tail = [2048, 1024, 512, 512] if taper and bi == BPC - 1 else []
    body = L - sum(head) - sum(tail)
    if body % tile_n:
        tail = [body % tile_n] + tail
        body -= body % tile_n
    widths = head + [tile_n] * (body // tile_n) + tail
    assert sum(widths) == L, widths
    return widths


def _build_nc(
    tile_n=TILE_N,
    bufs_in=BUFS_IN,
    bufs_mid=BUFS_MID,
    bufs_acc=BUFS_ACC,
    store_on_scalar=1,
    store_on_gpsimd=0,
    loads_alternate=0,
    store_skew=2,
    taper=1,
    repeat=1,
    memset_on_gpsimd=0,
    gpsimd_every=0,
    const_on_scalar=0,
    const_late=1,
    sub_n=SUB_N,
    bf16_tt=1,
    pool_tt_every=3,
    mix=0,
    bufs_t=3,
    bufs_p2=3,
    bufs_u=2,
    ndev=1,
):
    f32 = mybir.dt.float32
    nc = bacc.Bacc(
        "TRN2",
        target_bir_lowering=False,
        debug=False,
        enable_asserts=False,
        num_devices=ndev,
    )
    x = nc.dram_tensor("x", [BPC, C, L], f32, kind="ExternalInput").ap()
    w = nc.dram_tensor("w", [C, K], f32, kind="ExternalInput").ap()
    b = nc.dram_tensor("b", [C, 1], f32, kind="ExternalInput").ap()
    y = nc.dram_tensor("y", [BPC, C, L], f32, kind="ExternalOutput").ap()

    mult = mybir.AluOpType.mult
    add = mybir.AluOpType.add
    ident = mybir.ActivationFunctionType.Identity

    with tile.TileContext(nc) as tc:
        with (
            tc.tile_pool(name="const", bufs=1) as cpool,
            tc.tile_pool(name="work", bufs=1) as pool,
        ):
            wtile = cpool.tile([C, K], f32)
            btile = cpool.tile([C, 1], f32)
            const_eng = nc.scalar if const_on_scalar else nc.sync
            consts_emitted = [False]

            def _emit_consts():
                if not consts_emitted[0]:
                    const_eng.dma_start(out=wtile[:, :], in_=w)
                    const_eng.dma_start(out=btile[:, :], in_=b)
                    consts_emitted[0] = True

            if not const_late:
                _emit_consts()

            store_eng = nc.scalar if store_on_scalar else nc.sync
            if store_on_gpsimd:
                store_eng = nc.gpsimd
            memset_eng = nc.gpsimd if memset_on_gpsimd else nc.vector
            it = 0
            for rep in range(repeat):
              # pending (data_view, dram_view) stores, issued `store_skew`
              # chunks late so their semaphore wait is already satisfied
              # when the store ring reaches them
              pending = []
              for bi in range(BPC):
                l0 = 0
                for n in _row_widths(bi, tile_n, taper):
                    load_eng = (
                        nc.scalar if loads_alternate and (it % 2) else nc.sync
                    )
                    # input halo range [l0-1, l0+n+1) clipped to [0, L)
                    lo, hi = l0 - 1, l0 + n + 1
                    src_lo, src_hi = max(lo, 0), min(hi, L)
                    dst = src_lo - lo

                    xin = pool.tile([C, tile_n + 2], f32, tag="xin", bufs=bufs_in)
                    if lo < 0:
                        memset_eng.memset(xin[:, 0:1], 0.0)
                    if hi > L:
                        memset_eng.memset(xin[:, n + 1 : n + 2], 0.0)
                    load_eng.dma_start(
                        out=xin[:, dst : dst + (src_hi - src_lo)],
                        in_=x[bi, :, src_lo:src_hi],
                    )
                    _emit_consts()

                    stt_eng = (
                        nc.gpsimd
                        if gpsimd_every and (it % gpsimd_every == gpsimd_every - 1)
                        else nc.vector
                    )
                    # compute+store in sub_n-wide chunks (loads stay tile_n
                    # wide) to shorten the compute-to-store latency per byte
                    step = sub_n if sub_n and sub_n < n else n
                    for s0 in range(0, n, step):
                        sn = min(step, n - s0)
                        mid = pool.tile([C, step], f32, tag="mid", bufs=bufs_mid)
                        if mix:
                            # rotate chunk handling to balance ACT/DVE/Pool:
                            # A = f32 two-stt (light on ACT), B = bf16 tt on
                            # DVE, P = bf16 tt on Pool
                            ty = "BBPABPAP"[it % 8]
                            use_bf16 = ty != "A"
                            use_pool = ty == "P"
                        else:
                            use_bf16 = bool(bf16_tt)
                            use_pool = bool(
                                pool_tt_every
                                and (it % pool_tt_every == pool_tt_every - 1)
                            )
                        if use_bf16:
                            # ACT computes two taps in bf16; DVE combines
                            # them with a 2x-perf-mode bf16 tensor_tensor,
                            # then one stt adds the last tap in f32. This
                            # halves DVE's per-chunk time vs two f32 stt.
                            bf16 = mybir.dt.bfloat16
                            t = pool.tile([C, step], bf16, tag="t", bufs=bufs_t)
                            p2 = pool.tile([C, step], bf16, tag="p2", bufs=bufs_p2)
                            u = pool.tile([C, step], bf16, tag="u", bufs=bufs_u)
                            nc.scalar.activation(
                                t[:, 0:sn],
                                xin[:, s0 + 1 : s0 + sn + 1],
                                ident,
                                bias=btile[:, 0:1],
                                scale=wtile[:, 1:2],
                            )
                            nc.scalar.activation(
                                p2[:, 0:sn],
                                xin[:, s0 + 2 : s0 + sn + 2],
                                ident,
                                scale=wtile[:, 2:3],
                            )
                            tt_eng = nc.gpsimd if use_pool else nc.vector
                            tt_eng.tensor_tensor(
                                u[:, 0:sn], t[:, 0:sn], p2[:, 0:sn], add
                            )
                            nc.vector.scalar_tensor_tensor(
                                mid[:, 0:sn], xin[:, s0 : s0 + sn],
                                wtile[:, 0:1], u[:, 0:sn], mult, add
                            )
                        else:
                            acc = pool.tile(
                                [C, step], f32, tag="acc", bufs=bufs_acc
                            )
                            nc.scalar.activation(
                                mid[:, 0:sn],
                                xin[:, s0 + 1 : s0 + sn + 1],
                                ident,
                                bias=btile[:, 0:1],
                                scale=wtile[:, 1:2],
                            )
                            stt_eng.scalar_tensor_tensor(
                                acc[:, 0:sn], xin[:, s0 : s0 + sn],
                                wtile[:, 0:1], mid[:, 0:sn], mult, add
                            )
                            stt_eng.scalar_tensor_tensor(
                                mid[:, 0:sn], xin[:, s0 + 2 : s0 + sn + 2],
                                wtile[:, 2:3], acc[:, 0:sn], mult, add
                            )
                        pending.append(
                            (mid[:, 0:sn], y[bi, :, l0 + s0 : l0 + s0 + sn])
                        )
                        while len(pending) > store_skew:
                            src, dst = pending.pop(0)
                            store_eng.dma_start(out=dst, in_=src)
                    l0 += n
                    it += 1
              for src, dst in pending:
                  store_eng.dma_start(out=dst, in_=src)

    nc.compile()
    return nc


def _get_nc(**kw):
    key = tuple(sorted(kw.items()))
    if key not in _nc_cache:
        _nc_cache[key] = _build_nc(**kw)
    return _nc_cache[key]


def kernel_with_results(inputs, weight, bias, trace=False, **build_kw):
    x = np.ascontiguousarray(inputs, dtype=np.float32)
    w = np.ascontiguousarray(weight, dtype=np.float32)
    b = np.ascontiguousarray(bias, dtype=np.float32).reshape(C, 1)
    assert x.shape == (B, C, L), x.shape
    nc = _get_nc(**build_kw)
    in_maps = [
        {"x": x[i * BPC : (i + 1) * BPC], "w": w, "b": b} for i in range(NCORES)
    ]
    res = bass_utils.run_bass_kernel_spmd(
        nc, in_maps, core_ids=list(range(NCORES)), trace=trace
    )
    out = np.concatenate([r["y"] for r in res.results], axis=0)
    return out, res


def kernel(inputs, weight, bias):
    out, _ = kernel_with_results(inputs, weight, bias)
    return out



# revision 27
# speedup vs baseline: 1.2485x; 1.0987x over previous
"""Depthwise Conv1d (C=128, K=3, stride=1, pad=1) Trainium2 Bass kernel.

Layout: partitions = channels (C=128 exactly matches SBUF partitions).
Sharding: data-parallel over batch — 32 images / 8 cores = 4 images/core.
Per 4096-wide chunk (default bf16_tt path):
    ACT : t  = w1 * x_center + bias   (activation, f32 in -> bf16 out)
    ACT : p2 = w2 * x_right           (activation, f32 in -> bf16 out)
    DVE : u  = t + p2                 (tensor_tensor, all-bf16 -> 2x perf mode)
    DVE : res= (x_left * w0) + u      (scalar_tensor_tensor, f32 out)
    (every 3rd chunk's tensor_tensor runs on Pool/GpSimd instead of DVE)
bf16 intermediates round three of the four terms (~4e-3 rel error, well
inside the 2e-2 gate) and cut DVE's per-chunk time ~25% plus offload to
Pool, so compute stays off the critical path.
The kernel is HBM-bandwidth bound (~33.6 MB/core mandatory traffic;
measured sustained ~360-430 GB/s/core -> ~94us floor + ~9us NEFF
preamble). Stores issue on the scalar HWDGE ring two chunks late
(store_skew=2) so their semaphore wait is already satisfied and never
head-of-line-blocks the next activation; the final row tapers to small
tiles so the tail compute+store chain adds little to the DMA span.
"""

import numpy as np

import concourse.bacc as bacc
import concourse.mybir as mybir
import concourse.tile as tile
from concourse import bass_utils

B, C, L, K = 32, 128, 8192, 3
NCORES = 8
BPC = B // NCORES  # images per core

TILE_N = 4096
BUFS_IN = 5
BUFS_MID = 3
BUFS_ACC = 2
SUB_N = 4096

_nc_cache = {}


def _row_widths(bi, tile_n, taper):
    """Tile widths for image row bi (must sum to L)."""
    head = [512, 512, 1024, 2048] if taper == 2 and bi == 0 else []
    # shrink the final tiles so the tail dependency chain
    # (last load -> compute -> last store) is short; taper=3 uses a
    # coarser tail because the 4-op bf16 pipeline pays per-op dispatch
    # and semaphore latency on every extra tiny tile
    tail = [2048, 1024, 512, 512] if taper in (1, 2) and bi == BPC - 1 else []
    if taper == 3 and bi == BPC - 1:
        tail = [2048, 2048]
    body = L - sum(head) - sum(tail)
    if body % tile_n:
        tail = [body % tile_n] + tail
        body -= body % tile_n
    widths = head + [tile_n] * (body // tile_n) + tail
    assert sum(widths) == L, widths
    return widths


def _build_nc(
    tile_n=TILE_N,
    bufs_in=BUFS_IN,
    bufs_mid=BUFS_MID,
    bufs_acc=BUFS_ACC,
    store_on_scalar=1,
    store_on_gpsimd=0,
    loads_alternate=0,
    store_skew=2,
    taper=1,
    repeat=1,
    memset_on_gpsimd=0,
    gpsimd_every=0,
    const_on_scalar=0,
    const_late=1,
    sub_n=SUB_N,
    bf16_tt=1,
    pool_tt_every=3,
    mix=0,
    bufs_t=3,
    bufs_p2=3,
    bufs_u=2,
    ndev=1,
):
    f32 = mybir.dt.float32
    nc = bacc.Bacc(
        "TRN2",
        target_bir_lowering=False,
        debug=False,
        enable_asserts=False,
        num_devices=ndev,
    )
    x = nc.dram_tensor("x", [BPC, C, L], f32, kind="ExternalInput").ap()
    w = nc.dram_tensor("w", [C, K], f32, kind="ExternalInput").ap()
    b = nc.dram_tensor("b", [C, 1], f32, kind="ExternalInput").ap()
    y = nc.dram_tensor("y", [BPC, C, L], f32, kind="ExternalOutput").ap()

    mult = mybir.AluOpType.mult
    add = mybir.AluOpType.add
    ident = mybir.ActivationFunctionType.Identity

    with tile.TileContext(nc) as tc:
        with (
            tc.tile_pool(name="const", bufs=1) as cpool,
            tc.tile_pool(name="work", bufs=1) as pool,
        ):
            wtile = cpool.tile([C, K], f32)
            btile = cpool.tile([C, 1], f32)
            const_eng = nc.scalar if const_on_scalar else nc.sync
            consts_emitted = [False]

            def _emit_consts():
                if not consts_emitted[0]:
                    const_eng.dma_start(out=wtile[:, :], in_=w)
                    const_eng.dma_start(out=btile[:, :], in_=b)
                    consts_emitted[0] = True

            if not const_late:
                _emit_consts()

            store_eng = nc.scalar if store_on_scalar else nc.sync
            if store_on_gpsimd:
                store_eng = nc.gpsimd
            memset_eng = nc.gpsimd if memset_on_gpsimd else nc.vector
            it = 0
            for rep in range(repeat):
              # pending (data_view, dram_view) stores, issued `store_skew`
              # chunks late so their semaphore wait is already satisfied
              # when the store ring reaches them
              pending = []
              for bi in range(BPC):
                l0 = 0
                for n in _row_widths(bi, tile_n, taper):
                    load_eng = (
                        nc.scalar if loads_alternate and (it % 2) else nc.sync
                    )
                    # input halo range [l0-1, l0+n+1) clipped to [0, L)
                    lo, hi = l0 - 1, l0 + n + 1
                    src_lo, src_hi = max(lo, 0), min(hi, L)
                    dst = src_lo - lo

                    xin = pool.tile([C, tile_n + 2], f32, tag="xin", bufs=bufs_in)
                    if lo < 0:
                        memset_eng.memset(xin[:, 0:1], 0.0)
                    if hi > L:
                        memset_eng.memset(xin[:, n + 1 : n + 2], 0.0)
                    load_eng.dma_start(
                        out=xin[:, dst : dst + (src_hi - src_lo)],
                        in_=x[bi, :, src_lo:src_hi],
                    )
                    _emit_consts()

                    stt_eng = (
                        nc.gpsimd
                        if gpsimd_every and (it % gpsimd_every == gpsimd_every - 1)
                        else nc.vector
                    )
                    # compute+store in sub_n-wide chunks (loads stay tile_n
                    # wide) to shorten the compute-to-store latency per byte
                    step = sub_n if sub_n and sub_n < n else n
                    for s0 in range(0, n, step):
                        sn = min(step, n - s0)
                        mid = pool.tile([C, step], f32, tag="mid", bufs=bufs_mid)
                        if mix:
                            # rotate chunk handling to balance ACT/DVE/Pool:
                            # A = f32 two-stt (light on ACT), B = bf16 tt on
                            # DVE, P = bf16 tt on Pool
                            ty = "BBPABPAP"[it % 8]
                            use_bf16 = ty != "A"
                            use_pool = ty == "P"
                        else:
                            use_bf16 = bool(bf16_tt)
                            use_pool = bool(
                                pool_tt_every
                                and (it % pool_tt_every == pool_tt_every - 1)
                            )
                        if use_bf16:
                            # ACT computes two taps in bf16; DVE combines
                            # them with a 2x-perf-mode bf16 tensor_tensor,
                            # then one stt adds the last tap in f32. This
                            # halves DVE's per-chunk time vs two f32 stt.
                            bf16 = mybir.dt.bfloat16
                            t = pool.tile([C, step], bf16, tag="t", bufs=bufs_t)
                            p2 = pool.tile([C, step], bf16, tag="p2", bufs=bufs_p2)
                            u = pool.tile([C, step], bf16, tag="u", bufs=bufs_u)
                            nc.scalar.activation(
                                t[:, 0:sn],
                                xin[:, s0 + 1 : s0 + sn + 1],
                                ident,
                                bias=btile[:, 0:1],
                                scale=wtile[:, 1:2],
                            )
                            nc.scalar.activation(
                                p2[:, 0:sn],
                                xin[:, s0 + 2 : s0 + sn + 2],
                                ident,
                                scale=wtile[:, 2:3],
                            )
                            tt_eng = nc.gpsimd if use_pool else nc.vector
                            tt_eng.tensor_tensor(
                                u[:, 0:sn], t[:, 0:sn], p2[:, 0:sn], add
                            )
                            nc.vector.scalar_tensor_tensor(
                                mid[:, 0:sn], xin[:, s0 : s0 + sn],
                                wtile[:, 0:1], u[:, 0:sn], mult, add
                            )
                        else:
                            acc = pool.tile(
                                [C, step], f32, tag="acc", bufs=bufs_acc
                            )
                            nc.scalar.activation(
                                mid[:, 0:sn],
                                xin[:, s0 + 1 : s0 + sn + 1],
                                ident,
                                bias=btile[:, 0:1],
                                scale=wtile[:, 1:2],
                            )
                            stt_eng.scalar_tensor_tensor(
                                acc[:, 0:sn], xin[:, s0 : s0 + sn],
                                wtile[:, 0:1], mid[:, 0:sn], mult, add
                            )
                            stt_eng.scalar_tensor_tensor(
                                mid[:, 0:sn], xin[:, s0 + 2 : s0 + sn + 2],
                                wtile[:, 2:3], acc[:, 0:sn], mult, add
                            )
                        pending.append(
                            (mid[:, 0:sn], y[bi, :, l0 + s0 : l0 + s0 + sn])
                        )
                        while len(pending) > store_skew:
                            src, dst = pending.pop(0)
                            store_eng.dma_start(out=dst, in_=src)
                    l0 += n
                    it += 1
              for src, dst in pending:
                  store_eng.dma_start(out=dst, in_=src)

    nc.compile()
    return nc


def _get_nc(**kw):
    key = tuple(sorted(kw.items()))
    if key not in _nc_cache:
        _nc_cache[key] = _build_nc(**kw)
    return _nc_cache[key]


def kernel_with_results(inputs, weight, bias, trace=False, **build_kw):
    x = np.ascontiguousarray(inputs, dtype=np.float32)
    w = np.ascontiguousarray(weight, dtype=np.float32)
    b = np.ascontiguousarray(bias, dtype=np.float32).reshape(C, 1)
    assert x.shape == (B, C, L), x.shape
    nc = _get_nc(**build_kw)
    in_maps = [
        {"x": x[i * BPC : (i + 1) * BPC], "w": w, "b": b} for i in range(NCORES)
    ]
    res = bass_utils.run_bass_kernel_spmd(
        nc, in_maps, core_ids=list(range(NCORES)), trace=trace
    )
    out = np.concatenate([r["y"] for r in res.results], axis=0)
    return out, res


def kernel(inputs, weight, bias):
    out, _ = kernel_with_results(inputs, weight, bias)
    return out

